# revision 20
# baseline (speedup 1.0000x reference)
"""ComplexAttention Trainium2 kernel (Bass/Tile, SPMD over 8 NeuronCores).

Problem: complex-valued multi-head attention (B=2, N=2048, DIM=1024, 16 heads,
head_dim 64), fp32. See the reference:
  qkv = complex_linear(x, wqkv)           # 4 real matmuls + bias
  attn = softmax(Re(q . conj(k)) * scale) # contract head_dim AND real/imag
  out  = attn @ v (both channels)
  y    = complex_linear(out, wo)

Sharding (8 cores): data-parallel over batch (2) x tensor-parallel over head
groups (4 groups x 4 heads). Each core computes q/k/v for its 4 heads, full
attention for those heads, and a PARTIAL output projection (contraction over
its 256 of the 1024 concat features). The host sums the 4 partials per batch.

Numerics: all matmuls run as float32r (e8m11, ~2.4e-4 rel) at full PE rate
(1 cycle/row for moving dim >= 256 - 4x faster than plain fp32). Inputs are
pre-rounded to f32r on the host; PSUM accumulation is fp32.

Device data layouts (per core):
  xs    (2048, 2048) f32r  rows = [x_real.T (1024); x_imag.T (1024)], cols=tokens
  wq/wk (1024, 1024) f32r  cols = per head h: [A_h (128) | B_h (128)] where
                           A_h = [w_r_h; w_i_h].T, B_h = [-w_i_h; w_r_h].T
                           -> feature-major psum tiles [ (re_h 64; im_h 64), n ]
  wv    (1024, 768)  f32r  cols = [wv_r.T | -wv_i.T | wv_i.T] (token-major V)
  wo    (1024, 1024) f32r  rows 0:512 -> y_real coeffs, 512:1024 -> y_imag;
                           row order h*128 + c*64 + d matches AO layout
  qk_bias (128, 8) f32     per-partition bias columns [q h0..h3, k h0..h3]
  vbias (128, 512) f32     broadcast rows, cols [re bias 256 | im bias 256]
  obias (128, 2048) f32    broadcast rows [y_re 1024 | y_im 1024]; zero on g>0
Outputs: yr, yi (2048, 1024) f32 partial projections.

Softmax needs no max subtraction: scores are ~N(0,1) here (max |s| << 80).
Row sums come from a ones-column matmul chain; normalization is deferred to
after attn@V (divide the 128-row per-head output, not the 2048-row E matrix).

Pass 1 computes Q, K (feature-major) and V (token-major) for all tokens,
streaming x in 256-token blocks (f32r full-rate moving dim). Pass 2 runs
attention + projection with 512-wide tiles to amortize the per-matmul f32r
weight-load (~134ns).
"""

from contextlib import ExitStack

import numpy as np

import concourse.bacc as bacc
import concourse.mybir as mybir
import concourse.tile as tile
from concourse.bass_utils import run_bass_kernel_spmd

F32 = mybir.dt.float32
F32R = mybir.dt.float32r

B = 2
N = 2048
DIM = 1024
HEADS = 16
HD = 64
G = 4          # head groups (tensor-parallel factor)
HLOC = HEADS // G
SCALE = 1.0 / 8.0
NB1 = 256      # pass-1 token block
NB2 = 512      # pass-2 token block
DT = DIM // 128  # 8 contraction tiles per 1024
P = 128
MT = N // P    # 16 m-tiles

_CACHE = {}


def _round_f32r(a: np.ndarray) -> np.ndarray:
    """Round-to-nearest-even fp32 -> fp32r (e8m11: low 12 mantissa bits zero)."""
    v = np.ascontiguousarray(a, dtype=np.float32).view(np.uint32).copy()
    lsb = (v >> np.uint32(12)) & np.uint32(1)
    v = v + np.uint32(0x7FF) + lsb
    v &= np.uint32(0xFFFFF000)
    return v.view(np.float32)


def _build_program():
    nc = bacc.Bacc("TRN2", target_bir_lowering=False, debug=False, num_devices=8,
                   dynamic_dma_scratch_size=2048)

    xs = nc.dram_tensor("xs", [2 * DIM, N], F32R, kind="ExternalInput").ap()
    wq = nc.dram_tensor("wq", [DIM, 1024], F32R, kind="ExternalInput").ap()
    wk = nc.dram_tensor("wk", [DIM, 1024], F32R, kind="ExternalInput").ap()
    wv = nc.dram_tensor("wv", [DIM, 768], F32R, kind="ExternalInput").ap()
    wo = nc.dram_tensor("wo", [1024, 1024], F32R, kind="ExternalInput").ap()
    qkb_d = nc.dram_tensor("qk_bias", [P, 8], F32, kind="ExternalInput").ap()
    vb_d = nc.dram_tensor("vbias", [P, 512], F32, kind="ExternalInput").ap()
    ob_d = nc.dram_tensor("obias", [P, 2048], F32, kind="ExternalInput").ap()
    yr = nc.dram_tensor("yr", [N, 1024], F32, kind="ExternalOutput").ap()
    yi = nc.dram_tensor("yi", [N, 1024], F32, kind="ExternalOutput").ap()

    xs_r = xs.rearrange("(t p) m -> p t m", p=P)   # [128, 16, 2048]
    wq_r = wq.rearrange("(t p) c -> p t c", p=P)   # [128, 8, 1024]
    wk_r = wk.rearrange("(t p) c -> p t c", p=P)
    wv_r = wv.rearrange("(t p) c -> p t c", p=P)   # [128, 8, 768]
    wo_r = wo.rearrange("(t p) c -> p t c", p=P)   # [128, 8, 1024]

    with tile.TileContext(nc) as tc, ExitStack() as ctx:
        const = ctx.enter_context(tc.tile_pool(name="const", bufs=1))
        kvp = ctx.enter_context(tc.tile_pool(name="kv", bufs=1))

        onesc_f = const.tile([P, 1], F32)
        ones_col = const.tile([P, 1], F32R)
        nc.vector.memset(onesc_f[:], 1.0)
        nc.vector.tensor_copy(ones_col[:], onesc_f[:])
        qkb = const.tile([P, 8], F32)
        nc.sync.dma_start(out=qkb[:], in_=qkb_d[:])

        Q_sb = kvp.tile([P, HLOC, N], F32R)          # [comps, head, n]
        K_sb = kvp.tile([P, HLOC, N], F32R)          # [comps, head, m]
        # [m%128, mtile, head, c*64+d] - per-head [Vr_h | Vi_h] contiguous so
        # the PV stationary slice is a single free dim
        V_sb = kvp.tile([P, MT, HLOC, 128], F32R)

        p1 = ExitStack()
        xsp = p1.enter_context(tc.tile_pool(name="xs", bufs=3))
        # ---------------- pass 1: V first (small weight prereq), then K/Q ---
        # wk preloads during the V pass; wq loads during the first K chains
        # (Q chains lag K by one m-block so the load hides).
        with tc.tile_pool(name="wkp", bufs=1) as wkp:
            wk_t = []
            for dt in range(DT):
                wkt = wkp.tile([P, 1024], F32R, tag=f"wk{dt}", name=f"wk{dt}")
                wk_t.append(wkt)

            with tc.tile_pool(name="wvv", bufs=1) as wvvp:
                wv_t = []
                for dt in range(DT):
                    wvt = wvvp.tile([P, 768], F32R, tag=f"wv{dt}", name=f"wv{dt}")
                    nc.sync.dma_start(out=wvt[:], in_=wv_r[:, dt, :])
                    wv_t.append(wvt)
                vb = wvvp.tile([P, 512], F32)
                nc.sync.dma_start(out=vb[:], in_=vb_d[:])
                # first two x blocks ahead of the wk prefetch in the DMA queue
                xt_pre = []
                for mb in range(2):
                    xt = xsp.tile([P, 2 * DT, NB1], F32R, tag="xs",
                                  name=f"xtpre{mb}")
                    nc.sync.dma_start(
                        out=xt[:], in_=xs_r[:, :, mb * NB1:(mb + 1) * NB1])
                    xt_pre.append(xt)
                for dt in range(DT):
                    nc.sync.dma_start(out=wk_t[dt][:], in_=wk_r[:, dt, :])

                with tc.tile_pool(name="p1psb", bufs=6, space="PSUM") as pmm:
                    for mb in range(N // NB1):
                        if mb < 2:
                            xt = xt_pre[mb]
                        else:
                            xt = xsp.tile([P, 2 * DT, NB1], F32R, tag="xs")
                            nc.sync.dma_start(
                                out=xt[:], in_=xs_r[:, :, mb * NB1:(mb + 1) * NB1])
                        for mt in range(NB1 // P):
                            mtg = mb * (NB1 // P) + mt
                            for c in range(2):
                                kind_a = 0 if c == 0 else 2   # wv_r.T / wv_i.T
                                kind_b = 1 if c == 0 else 0   # -wv_i.T / wv_r.T
                                ps = pmm.tile([P, NB2], F32, tag="mm")
                                for dt in range(DT):
                                    nc.tensor.matmul(
                                        ps[:, :256], xt[:, dt, mt * P:(mt + 1) * P],
                                        wv_t[dt][:, kind_a * 256:(kind_a + 1) * 256],
                                        start=(dt == 0), stop=False)
                                for dt in range(DT):
                                    nc.tensor.matmul(
                                        ps[:, :256],
                                        xt[:, DT + dt, mt * P:(mt + 1) * P],
                                        wv_t[dt][:, kind_b * 256:(kind_b + 1) * 256],
                                        start=False, stop=(dt == DT - 1))
                                nc.vector.tensor_add(
                                    V_sb[:, mtg, :, c * HD:(c + 1) * HD],
                                    ps[:, :256].rearrange("p (h f) -> p h f", f=HD),
                                    vb[:, c * 256:(c + 1) * 256].rearrange(
                                        "p (h f) -> p h f", f=HD))

            # ---- K/Q: third x stream; Q lags K by one block -----------------
            with tc.tile_pool(name="wqp2", bufs=1) as wqp2, \
                 tc.tile_pool(name="p1ps", bufs=6, space="PSUM") as pmm:
                wq_t = []
                for dt in range(DT):
                    wqt = wqp2.tile([P, 1024], F32R, tag=f"wq{dt}", name=f"wq{dt}")
                    nc.sync.dma_start(out=wqt[:], in_=wq_r[:, dt, :])
                    wq_t.append(wqt)

                def kq_chains(w_t, sb, bcol, h, xt, mb):
                    ps = pmm.tile([P, NB2], F32, tag="mm")
                    for dt in range(DT):
                        nc.tensor.matmul(
                            ps[:, :NB1], w_t[dt][:, h * 256:h * 256 + 128],
                            xt[:, dt, :], start=(dt == 0), stop=False)
                    for dt in range(DT):
                        nc.tensor.matmul(
                            ps[:, :NB1], w_t[dt][:, h * 256 + 128:h * 256 + 256],
                            xt[:, DT + dt, :], start=False, stop=(dt == DT - 1))
                    nc.vector.tensor_scalar_add(
                        sb[:, h, mb * NB1:(mb + 1) * NB1], ps[:, :NB1],
                        qkb[:, bcol + h:bcol + h + 1])

                prev_xt = None
                for mb in range(N // NB1):
                    xt = xsp.tile([P, 2 * DT, NB1], F32R, tag="xs")
                    nc.sync.dma_start(
                        out=xt[:], in_=xs_r[:, :, mb * NB1:(mb + 1) * NB1])
                    for h in range(HLOC):
                        kq_chains(wk_t, K_sb, 4, h, xt, mb)
                    if prev_xt is not None:
                        for h in range(HLOC):
                            kq_chains(wq_t, Q_sb, 0, h, prev_xt[0], prev_xt[1])
                    prev_xt = (xt, mb)
                for h in range(HLOC):
                    kq_chains(wq_t, Q_sb, 0, h, prev_xt[0], prev_xt[1])

        p1.close()

        # ---------------- pass 2: attention + partial out-projection -------
        with tc.tile_pool(name="w2", bufs=1) as w2p, \
             tc.tile_pool(name="pmm2", bufs=3, space="PSUM") as pmm, \
             tc.tile_pool(name="ppv", bufs=1, space="PSUM") as ppv, \
             tc.tile_pool(name="prs", bufs=1, space="PSUM") as prs, \
             tc.tile_pool(name="epool", bufs=6) as ep, \
             tc.tile_pool(name="aop", bufs=2) as aop, \
             tc.tile_pool(name="rbp", bufs=4) as rbp, \
             tc.tile_pool(name="outp", bufs=6) as outp:
            wo_sb = w2p.tile([P, 8, 1024], F32R)
            ob = w2p.tile([P, 2048], F32)
            nc.sync.dma_start(out=wo_sb[:], in_=wo_r[:])
            nc.sync.dma_start(out=ob[:], in_=ob_d[:])

            def emit_proj(ao, nb):
                for ns in range(NB2 // P):
                    for half in range(2):
                        for ri, (ydram, bofs) in enumerate(((yr, 0), (yi, 1024))):
                            pp = pmm.tile([P, 512], F32, tag="mm")
                            for t in range(4):
                                nc.tensor.matmul(
                                    pp[:], ao[:, t, ns * P:(ns + 1) * P],
                                    wo_sb[:, 4 * ri + t, half * 512:(half + 1) * 512],
                                    start=(t == 0), stop=(t == 3))
                            ot = outp.tile([P, 512], F32, tag="ot")
                            nc.vector.tensor_add(
                                ot[:], pp[:],
                                ob[:, bofs + half * 512:bofs + (half + 1) * 512])
                            nc.sync.dma_start(
                                out=ydram[nb * NB2 + ns * P:nb * NB2 + (ns + 1) * P,
                                          half * 512:(half + 1) * 512],
                                in_=ot[:])

            prev = None
            for nb in range(N // NB2):
                ao = aop.tile([P, HLOC, NB2], F32R)
                for h in range(HLOC):
                    pv = ppv.tile([P, NB2], F32, tag="pv")
                    rs = prs.tile([1, NB2], F32, tag="rs")
                    # software pipeline: scores one pair ahead of PV/rowsum so
                    # the PE never stalls on the exp latency
                    pend = None
                    for mtp in range(MT // 2):
                        mt0, mt1 = 2 * mtp, 2 * mtp + 1
                        ss = pmm.tile([P, 2 * NB2], F32, tag="mm")
                        nc.tensor.matmul(
                            ss[:, :NB2], K_sb[:, h, mt0 * P:(mt0 + 1) * P],
                            Q_sb[:, h, nb * NB2:(nb + 1) * NB2],
                            start=True, stop=True)
                        nc.tensor.matmul(
                            ss[:, NB2:], K_sb[:, h, mt1 * P:(mt1 + 1) * P],
                            Q_sb[:, h, nb * NB2:(nb + 1) * NB2],
                            start=True, stop=True)
                        e = ep.tile([P, 2 * NB2], F32R, tag="e")
                        nc.scalar.activation(
                            e[:], ss[:], mybir.ActivationFunctionType.Exp,
                            scale=SCALE)
                        if pend is not None:
                            pe, pmt = pend
                            nc.tensor.matmul(
                                pv[:], V_sb[:, 2 * pmt, h, :], pe[:, :NB2],
                                start=(pmt == 0), stop=False)
                            nc.tensor.matmul(
                                pv[:], V_sb[:, 2 * pmt + 1, h, :], pe[:, NB2:],
                                start=False, stop=False)
                            nc.tensor.matmul(
                                rs[:], ones_col[:], pe[:, :NB2],
                                start=(pmt == 0), stop=False)
                            nc.tensor.matmul(
                                rs[:], ones_col[:], pe[:, NB2:],
                                start=False, stop=False)
                        pend = (e, mtp)
                    pe, pmt = pend
                    nc.tensor.matmul(
                        pv[:], V_sb[:, 2 * pmt, h, :], pe[:, :NB2],
                        start=False, stop=False)
                    nc.tensor.matmul(
                        pv[:], V_sb[:, 2 * pmt + 1, h, :], pe[:, NB2:],
                        start=False, stop=True)
                    nc.tensor.matmul(
                        rs[:], ones_col[:], pe[:, :NB2],
                        start=False, stop=False)
                    nc.tensor.matmul(
                        rs[:], ones_col[:], pe[:, NB2:],
                        start=False, stop=True)
                    rsc = rbp.tile([1, NB2], F32, tag="rsc", bufs=2)
                    nc.vector.tensor_copy(rsc[:], rs[:])
                    # release pv early: stage unnormalized output into ao, then
                    # scale in place once the reciprocal lands
                    nc.vector.tensor_copy(ao[:, h, :], pv[:])
                    rbr = rbp.tile([P, NB2], F32, tag="rbr")
                    nc.gpsimd.partition_broadcast(rbr[:], rsc[:])
                    rbs = rbp.tile([P, NB2], F32, tag="rb")
                    with nc.allow_low_precision(reason="softmax divisor"):
                        nc.vector.reciprocal(rbs[:], rbr[:])
                    nc.vector.tensor_mul(ao[:, h, :], ao[:, h, :], rbs[:])
                    if h == 0 and prev is not None:
                        emit_proj(*prev)
                prev = (ao, nb)
            emit_proj(*prev)
    nc.compile()
    return nc


def _prepare_in_maps(x, wqkv_r, wqkv_i, bqkv_r, bqkv_i, wo_r, wo_i, bo_r, bo_i):
    x = np.asarray(x, np.float32)
    wqkv_r = np.asarray(wqkv_r, np.float32)
    wqkv_i = np.asarray(wqkv_i, np.float32)
    bqkv_r = np.asarray(bqkv_r, np.float32)
    bqkv_i = np.asarray(bqkv_i, np.float32)
    wo_r = np.asarray(wo_r, np.float32)
    wo_i = np.asarray(wo_i, np.float32)
    bo_r = np.asarray(bo_r, np.float32)
    bo_i = np.asarray(bo_i, np.float32)

    bdiff = bqkv_r - bqkv_i
    bsum = bqkv_r + bqkv_i

    xs_by_b = []
    for b in range(B):
        xs_by_b.append(_round_f32r(np.concatenate(
            [np.ascontiguousarray(x[b, :, :, 0].T),
             np.ascontiguousarray(x[b, :, :, 1].T)], axis=0)))

    per_g = []
    for g in range(G):
        # q/k weights: per head, stacked-complex A/B column blocks
        def head_cols(base):
            cols = []
            for h in range(HLOC):
                rows = slice(base + g * 256 + h * HD, base + g * 256 + (h + 1) * HD)
                a = np.concatenate([wqkv_r[rows], wqkv_i[rows]], axis=0).T
                bb = np.concatenate([-wqkv_i[rows], wqkv_r[rows]], axis=0).T
                cols.append(a)
                cols.append(bb)
            return _round_f32r(np.concatenate(cols, axis=1))  # (1024, 1024)

        wq_host = head_cols(0)
        wk_host = head_cols(DIM)
        vrows = slice(2 * DIM + g * 256, 2 * DIM + (g + 1) * 256)
        wv_host = _round_f32r(np.concatenate(
            [wqkv_r[vrows].T, -wqkv_i[vrows].T, wqkv_i[vrows].T], axis=1))

        cols_g = slice(g * 256, (g + 1) * 256)
        wotr = np.ascontiguousarray(wo_r[:, cols_g].T)   # (256 fi, 1024 fo)
        woti = np.ascontiguousarray(wo_i[:, cols_g].T)
        yr_blk = np.concatenate(
            [wotr.reshape(HLOC, HD, 1024), -woti.reshape(HLOC, HD, 1024)],
            axis=1).reshape(512, 1024)
        yi_blk = np.concatenate(
            [woti.reshape(HLOC, HD, 1024), wotr.reshape(HLOC, HD, 1024)],
            axis=1).reshape(512, 1024)
        wo_host = _round_f32r(np.concatenate([yr_blk, yi_blk], axis=0))

        qkb = np.zeros((P, 8), np.float32)
        for h in range(HLOC):
            qrows = slice(g * 256 + h * HD, g * 256 + (h + 1) * HD)
            krows = slice(DIM + g * 256 + h * HD, DIM + g * 256 + (h + 1) * HD)
            qkb[:, h] = np.concatenate([bdiff[qrows], bsum[qrows]])
            qkb[:, 4 + h] = np.concatenate([bdiff[krows], bsum[krows]])
        vbias = np.broadcast_to(
            np.concatenate([bdiff[vrows], bsum[vrows]]), (P, 512)).copy()
        if g == 0:
            obias = np.concatenate(
                [np.broadcast_to(bo_r - bo_i, (P, 1024)),
                 np.broadcast_to(bo_r + bo_i, (P, 1024))], axis=1).astype(np.float32)
        else:
            obias = np.zeros((P, 2048), np.float32)
        per_g.append((wq_host, wk_host, wv_host, wo_host, qkb, vbias,
                      np.ascontiguousarray(obias)))

    in_maps = []
    for core in range(8):
        b, g = divmod(core, G)
        wq_host, wk_host, wv_host, wo_host, qkb, vbias, obias = per_g[g]
        in_maps.append({
            "xs": xs_by_b[b], "wq": wq_host, "wk": wk_host, "wv": wv_host,
            "wo": wo_host, "qk_bias": qkb, "vbias": vbias, "obias": obias,
        })
    return in_maps


def _get_program():
    if "nc" not in _CACHE:
        _CACHE["nc"] = _build_program()
    return _CACHE["nc"]


def run(inputs: dict, trace: bool = False):
    """Returns (output, BassKernelResults)."""
    nc = _get_program()
    in_maps = _prepare_in_maps(**inputs)
    res = run_bass_kernel_spmd(nc, in_maps, list(range(8)), trace=trace)
    out = np.zeros((B, N, DIM, 2), np.float64)
    for core in range(8):
        b = core // G
        out[b, :, :, 0] += res.results[core]["yr"]
        out[b, :, :, 1] += res.results[core]["yi"]
    return out.astype(np.float32), res


def kernel(**inputs) -> np.ndarray:
    out, _ = run(inputs)
    return out


# revision 21
# speedup vs baseline: 1.1727x; 1.1727x over previous
"""ComplexAttention Trainium2 kernel (Bass/Tile, SPMD over 8 NeuronCores).

Problem: complex-valued multi-head attention (B=2, N=2048, DIM=1024, 16 heads,
head_dim 64), fp32. See the reference:
  qkv = complex_linear(x, wqkv)           # 4 real matmuls + bias
  attn = softmax(Re(q . conj(k)) * scale) # contract head_dim AND real/imag
  out  = attn @ v (both channels)
  y    = complex_linear(out, wo)

Sharding (8 cores): data-parallel over batch (2) x tensor-parallel over head
groups (4 groups x 4 heads). Each core computes q/k/v for its 4 heads, full
attention for those heads, and a PARTIAL output projection (contraction over
its 256 of the 1024 concat features). The host sums the 4 partials per batch.

Numerics: all matmuls run as float32r (e8m11, ~2.4e-4 rel) at full PE rate
(1 cycle/row for moving dim >= 256 - 4x faster than plain fp32). Inputs are
pre-rounded to f32r on the host; PSUM accumulation is fp32.

Device data layouts (per core):
  xs    (2048, 2048) f32r  rows = [x_real.T (1024); x_imag.T (1024)], cols=tokens
  wq/wk (1024, 1024) f32r  cols = per head h: [A_h (128) | B_h (128)] where
                           A_h = [w_r_h; w_i_h].T, B_h = [-w_i_h; w_r_h].T
                           -> feature-major psum tiles [ (re_h 64; im_h 64), n ]
  wv    (1024, 768)  f32r  cols = [wv_r.T | -wv_i.T | wv_i.T] (token-major V)
  wo    (1024, 1024) f32r  rows 0:512 -> y_real coeffs, 512:1024 -> y_imag;
                           row order h*128 + c*64 + d matches AO layout
  qk_bias (128, 8) f32     per-partition bias columns [q h0..h3, k h0..h3]
  vbias (128, 512) f32     broadcast rows, cols [re bias 256 | im bias 256]
  obias (128, 2048) f32    broadcast rows [y_re 1024 | y_im 1024]; zero on g>0
Outputs: yr, yi (2048, 1024) f32 partial projections.

Softmax needs no max subtraction: scores are ~N(0,1) here (max |s| << 80).
Row sums come from a ones-column matmul chain; normalization is deferred to
after attn@V (divide the 128-row per-head output, not the 2048-row E matrix).

Pass 1 computes Q, K (feature-major) and V (token-major) for all tokens,
streaming x in 256-token blocks (f32r full-rate moving dim). Pass 2 runs
attention + projection with 512-wide tiles to amortize the per-matmul f32r
weight-load (~134ns).
"""

from contextlib import ExitStack

import numpy as np

import concourse.bacc as bacc
import concourse.mybir as mybir
import concourse.tile as tile
from concourse.bass_utils import run_bass_kernel_spmd

F32 = mybir.dt.float32
F32R = mybir.dt.float32r

B = 2
N = 2048
DIM = 1024
HEADS = 16
HD = 64
G = 4          # head groups (tensor-parallel factor)
HLOC = HEADS // G
SCALE = 1.0 / 8.0
NB1 = 256      # pass-1 token block
NB2 = 512      # pass-2 token block
DT = DIM // 128  # 8 contraction tiles per 1024
P = 128
MT = N // P    # 16 m-tiles

_CACHE = {}


def _round_f32r(a: np.ndarray) -> np.ndarray:
    """Round-to-nearest-even fp32 -> fp32r (e8m11: low 12 mantissa bits zero)."""
    v = np.ascontiguousarray(a, dtype=np.float32).view(np.uint32).copy()
    lsb = (v >> np.uint32(12)) & np.uint32(1)
    v = v + np.uint32(0x7FF) + lsb
    v &= np.uint32(0xFFFFF000)
    return v.view(np.float32)


def _build_program():
    nc = bacc.Bacc("TRN2", target_bir_lowering=False, debug=False, num_devices=8,
                   dynamic_dma_scratch_size=2048)

    xs = nc.dram_tensor("xs", [2 * DIM, N], F32R, kind="ExternalInput").ap()
    wq = nc.dram_tensor("wq", [DIM, 1024], F32R, kind="ExternalInput").ap()
    wk = nc.dram_tensor("wk", [DIM, 1024], F32R, kind="ExternalInput").ap()
    wv = nc.dram_tensor("wv", [DIM, 768], F32R, kind="ExternalInput").ap()
    wo = nc.dram_tensor("wo", [1024, 1024], F32R, kind="ExternalInput").ap()
    qkb_d = nc.dram_tensor("qk_bias", [P, 8], F32, kind="ExternalInput").ap()
    vb_d = nc.dram_tensor("vbias", [P, 512], F32, kind="ExternalInput").ap()
    ob_d = nc.dram_tensor("obias", [P, 2048], F32, kind="ExternalInput").ap()
    yr = nc.dram_tensor("yr", [N, 1024], F32, kind="ExternalOutput").ap()
    yi = nc.dram_tensor("yi", [N, 1024], F32, kind="ExternalOutput").ap()

    xs_r = xs.rearrange("(t p) m -> p t m", p=P)   # [128, 16, 2048]
    wq_r = wq.rearrange("(t p) c -> p t c", p=P)   # [128, 8, 1024]
    wk_r = wk.rearrange("(t p) c -> p t c", p=P)
    wv_r = wv.rearrange("(t p) c -> p t c", p=P)   # [128, 8, 768]
    wo_r = wo.rearrange("(t p) c -> p t c", p=P)   # [128, 8, 1024]

    with tile.TileContext(nc) as tc, ExitStack() as ctx:
        const = ctx.enter_context(tc.tile_pool(name="const", bufs=1))
        kvp = ctx.enter_context(tc.tile_pool(name="kv", bufs=1))

        onesc_f = const.tile([P, 1], F32)
        ones_col = const.tile([P, 1], F32R)
        nc.vector.memset(onesc_f[:], 1.0)
        nc.vector.tensor_copy(ones_col[:], onesc_f[:])
        qkb = const.tile([P, 8], F32)
        nc.sync.dma_start(out=qkb[:], in_=qkb_d[:])

        Q_sb = kvp.tile([P, HLOC, N], F32R)          # [comps, head, n]
        K_sb = kvp.tile([P, HLOC, N], F32R)          # [comps, head, m]
        # [m%128, mtile, head, c*64+d] - per-head [Vr_h | Vi_h] contiguous so
        # the PV stationary slice is a single free dim
        V_sb = kvp.tile([P, MT, HLOC, 128], F32R)

        p1 = ExitStack()
        xsp = p1.enter_context(tc.tile_pool(name="xs", bufs=3))
        # ---------------- pass 1: V first (small weight prereq), then K/Q ---
        # wk preloads during the V pass; wq loads during the first K chains
        # (Q chains lag K by one m-block so the load hides).
        with tc.tile_pool(name="wkp", bufs=1) as wkp:
            wk_t = []
            for dt in range(DT):
                wkt = wkp.tile([P, 1024], F32R, tag=f"wk{dt}", name=f"wk{dt}")
                wk_t.append(wkt)

            with tc.tile_pool(name="wvv", bufs=1) as wvvp:
                wv_t = []
                for dt in range(DT):
                    wvt = wvvp.tile([P, 768], F32R, tag=f"wv{dt}", name=f"wv{dt}")
                    nc.sync.dma_start(out=wvt[:], in_=wv_r[:, dt, :])
                    wv_t.append(wvt)
                vb = wvvp.tile([P, 512], F32)
                nc.sync.dma_start(out=vb[:], in_=vb_d[:])
                # first two x blocks ahead of the wk prefetch in the DMA queue
                xt_pre = []
                for mb in range(2):
                    xt = xsp.tile([P, 2 * DT, NB1], F32R, tag="xs",
                                  name=f"xtpre{mb}")
                    nc.sync.dma_start(
                        out=xt[:], in_=xs_r[:, :, mb * NB1:(mb + 1) * NB1])
                    xt_pre.append(xt)
                for dt in range(DT):
                    nc.sync.dma_start(out=wk_t[dt][:], in_=wk_r[:, dt, :])

                with tc.tile_pool(name="p1psb", bufs=6, space="PSUM") as pmm:
                    for mb in range(N // NB1):
                        if mb < 2:
                            xt = xt_pre[mb]
                        else:
                            xt = xsp.tile([P, 2 * DT, NB1], F32R, tag="xs")
                            nc.sync.dma_start(
                                out=xt[:], in_=xs_r[:, :, mb * NB1:(mb + 1) * NB1])
                        for mt in range(NB1 // P):
                            mtg = mb * (NB1 // P) + mt
                            for c in range(2):
                                kind_a = 0 if c == 0 else 2   # wv_r.T / wv_i.T
                                kind_b = 1 if c == 0 else 0   # -wv_i.T / wv_r.T
                                ps = pmm.tile([P, NB2], F32, tag="mm")
                                for dt in range(DT):
                                    nc.tensor.matmul(
                                        ps[:, :256], xt[:, dt, mt * P:(mt + 1) * P],
                                        wv_t[dt][:, kind_a * 256:(kind_a + 1) * 256],
                                        start=(dt == 0), stop=False)
                                for dt in range(DT):
                                    nc.tensor.matmul(
                                        ps[:, :256],
                                        xt[:, DT + dt, mt * P:(mt + 1) * P],
                                        wv_t[dt][:, kind_b * 256:(kind_b + 1) * 256],
                                        start=False, stop=(dt == DT - 1))
                                nc.vector.tensor_add(
                                    V_sb[:, mtg, :, c * HD:(c + 1) * HD],
                                    ps[:, :256].rearrange("p (h f) -> p h f", f=HD),
                                    vb[:, c * 256:(c + 1) * 256].rearrange(
                                        "p (h f) -> p h f", f=HD))

            # ---- K/Q: third x stream; Q lags K by one block -----------------
            with tc.tile_pool(name="wqp2", bufs=1) as wqp2, \
                 tc.tile_pool(name="p1ps", bufs=6, space="PSUM") as pmm:
                wq_t = []
                for dt in range(DT):
                    wqt = wqp2.tile([P, 1024], F32R, tag=f"wq{dt}", name=f"wq{dt}")
                    nc.sync.dma_start(out=wqt[:], in_=wq_r[:, dt, :])
                    wq_t.append(wqt)

                def kq_chains(w_t, sb, bcol, h, xt, mb):
                    ps = pmm.tile([P, NB2], F32, tag="mm")
                    for dt in range(DT):
                        nc.tensor.matmul(
                            ps[:, :NB1], w_t[dt][:, h * 256:h * 256 + 128],
                            xt[:, dt, :], start=(dt == 0), stop=False)
                    for dt in range(DT):
                        nc.tensor.matmul(
                            ps[:, :NB1], w_t[dt][:, h * 256 + 128:h * 256 + 256],
                            xt[:, DT + dt, :], start=False, stop=(dt == DT - 1))
                    nc.vector.tensor_scalar_add(
                        sb[:, h, mb * NB1:(mb + 1) * NB1], ps[:, :NB1],
                        qkb[:, bcol + h:bcol + h + 1])

                prev_xt = None
                for mb in range(N // NB1):
                    xt = xsp.tile([P, 2 * DT, NB1], F32R, tag="xs")
                    nc.sync.dma_start(
                        out=xt[:], in_=xs_r[:, :, mb * NB1:(mb + 1) * NB1])
                    for h in range(HLOC):
                        kq_chains(wk_t, K_sb, 4, h, xt, mb)
                    if prev_xt is not None:
                        for h in range(HLOC):
                            kq_chains(wq_t, Q_sb, 0, h, prev_xt[0], prev_xt[1])
                    prev_xt = (xt, mb)
                for h in range(HLOC):
                    kq_chains(wq_t, Q_sb, 0, h, prev_xt[0], prev_xt[1])

        p1.close()

        # ---------------- pass 2: attention + partial out-projection -------
        with tc.tile_pool(name="w2", bufs=1) as w2p, \
             tc.tile_pool(name="pmm2", bufs=3, space="PSUM") as pmm, \
             tc.tile_pool(name="ppv", bufs=1, space="PSUM") as ppv, \
             tc.tile_pool(name="prs", bufs=1, space="PSUM") as prs, \
             tc.tile_pool(name="epool", bufs=4) as ep, \
             tc.tile_pool(name="aop", bufs=2) as aop, \
             tc.tile_pool(name="rbp", bufs=4) as rbp, \
             tc.tile_pool(name="outp", bufs=4) as outp:
            wo_sb = w2p.tile([P, 8, 1024], F32R)
            ob = w2p.tile([P, 2048], F32)
            nc.sync.dma_start(out=wo_sb[:], in_=wo_r[:])
            nc.sync.dma_start(out=ob[:], in_=ob_d[:])

            def emit_proj(ao, nb):
                for ns in range(NB2 // P):
                    for half in range(2):
                        for ri, (ydram, bofs) in enumerate(((yr, 0), (yi, 1024))):
                            pp = pmm.tile([P, 512], F32, tag="mm")
                            for t in range(4):
                                nc.tensor.matmul(
                                    pp[:], ao[:, t, ns * P:(ns + 1) * P],
                                    wo_sb[:, 4 * ri + t, half * 512:(half + 1) * 512],
                                    start=(t == 0), stop=(t == 3))
                            ot = outp.tile([P, 512], F32, tag="ot")
                            nc.vector.tensor_add(
                                ot[:], pp[:],
                                ob[:, bofs + half * 512:bofs + (half + 1) * 512])
                            nc.sync.dma_start(
                                out=ydram[nb * NB2 + ns * P:nb * NB2 + (ns + 1) * P,
                                          half * 512:(half + 1) * 512],
                                in_=ot[:])

            prev = None
            for nb in range(N // NB2):
                ao = aop.tile([P, HLOC, NB2], F32R)
                for h in range(HLOC):
                    pv = ppv.tile([P, NB2], F32, tag="pv")
                    rs = prs.tile([1, NB2], F32, tag="rs")
                    # software pipeline: scores one pair ahead of PV/rowsum so
                    # the PE never stalls on the exp latency
                    pend = None
                    for mtp in range(MT // 2):
                        mt0, mt1 = 2 * mtp, 2 * mtp + 1
                        ss = pmm.tile([P, 2 * NB2], F32, tag="mm")
                        nc.tensor.matmul(
                            ss[:, :NB2], K_sb[:, h, mt0 * P:(mt0 + 1) * P],
                            Q_sb[:, h, nb * NB2:(nb + 1) * NB2],
                            start=True, stop=True)
                        nc.tensor.matmul(
                            ss[:, NB2:], K_sb[:, h, mt1 * P:(mt1 + 1) * P],
                            Q_sb[:, h, nb * NB2:(nb + 1) * NB2],
                            start=True, stop=True)
                        e = ep.tile([P, 2 * NB2], F32R, tag="e")
                        nc.scalar.activation(
                            e[:], ss[:], mybir.ActivationFunctionType.Exp,
                            scale=SCALE)
                        if pend is not None:
                            pe, pmt = pend
                            nc.tensor.matmul(
                                pv[:], V_sb[:, 2 * pmt, h, :], pe[:, :NB2],
                                start=(pmt == 0), stop=False)
                            nc.tensor.matmul(
                                pv[:], V_sb[:, 2 * pmt + 1, h, :], pe[:, NB2:],
                                start=False, stop=False)
                            nc.tensor.matmul(
                                rs[:], ones_col[:], pe[:, :NB2],
                                start=(pmt == 0), stop=False)
                            nc.tensor.matmul(
                                rs[:], ones_col[:], pe[:, NB2:],
                                start=False, stop=False)
                        pend = (e, mtp)
                    pe, pmt = pend
                    nc.tensor.matmul(
                        pv[:], V_sb[:, 2 * pmt, h, :], pe[:, :NB2],
                        start=False, stop=False)
                    nc.tensor.matmul(
                        pv[:], V_sb[:, 2 * pmt + 1, h, :], pe[:, NB2:],
                        start=False, stop=True)
                    nc.tensor.matmul(
                        rs[:], ones_col[:], pe[:, :NB2],
                        start=False, stop=False)
                    nc.tensor.matmul(
                        rs[:], ones_col[:], pe[:, NB2:],
                        start=False, stop=True)
                    rsc = rbp.tile([1, NB2], F32, tag="rsc", bufs=2)
                    nc.vector.tensor_copy(rsc[:], rs[:])
                    # release pv early: stage unnormalized output into ao, then
                    # scale in place once the reciprocal lands
                    nc.vector.tensor_copy(ao[:, h, :], pv[:])
                    rbr = rbp.tile([P, NB2], F32, tag="rbr")
                    nc.gpsimd.partition_broadcast(rbr[:], rsc[:])
                    rbs = rbp.tile([P, NB2], F32, tag="rb")
                    with nc.allow_low_precision(reason="softmax divisor"):
                        nc.vector.reciprocal(rbs[:], rbr[:])
                    nc.vector.tensor_mul(ao[:, h, :], ao[:, h, :], rbs[:])
                    if h == 0 and prev is not None:
                        emit_proj(*prev)
                prev = (ao, nb)
            emit_proj(*prev)
    nc.compile()
    return nc


def _prepare_in_maps(x, wqkv_r, wqkv_i, bqkv_r, bqkv_i, wo_r, wo_i, bo_r, bo_i):
    x = np.asarray(x, np.float32)
    wqkv_r = np.asarray(wqkv_r, np.float32)
    wqkv_i = np.asarray(wqkv_i, np.float32)
    bqkv_r = np.asarray(bqkv_r, np.float32)
    bqkv_i = np.asarray(bqkv_i, np.float32)
    wo_r = np.asarray(wo_r, np.float32)
    wo_i = np.asarray(wo_i, np.float32)
    bo_r = np.asarray(bo_r, np.float32)
    bo_i = np.asarray(bo_i, np.float32)

    bdiff = bqkv_r - bqkv_i
    bsum = bqkv_r + bqkv_i

    xs_by_b = []
    for b in range(B):
        xs_by_b.append(_round_f32r(np.concatenate(
            [np.ascontiguousarray(x[b, :, :, 0].T),
             np.ascontiguousarray(x[b, :, :, 1].T)], axis=0)))

    per_g = []
    for g in range(G):
        # q/k weights: per head, stacked-complex A/B column blocks
        def head_cols(base):
            cols = []
            for h in range(HLOC):
                rows = slice(base + g * 256 + h * HD, base + g * 256 + (h + 1) * HD)
                a = np.concatenate([wqkv_r[rows], wqkv_i[rows]], axis=0).T
                bb = np.concatenate([-wqkv_i[rows], wqkv_r[rows]], axis=0).T
                cols.append(a)
                cols.append(bb)
            return _round_f32r(np.concatenate(cols, axis=1))  # (1024, 1024)

        wq_host = head_cols(0)
        wk_host = head_cols(DIM)
        vrows = slice(2 * DIM + g * 256, 2 * DIM + (g + 1) * 256)
        wv_host = _round_f32r(np.concatenate(
            [wqkv_r[vrows].T, -wqkv_i[vrows].T, wqkv_i[vrows].T], axis=1))

        cols_g = slice(g * 256, (g + 1) * 256)
        wotr = np.ascontiguousarray(wo_r[:, cols_g].T)   # (256 fi, 1024 fo)
        woti = np.ascontiguousarray(wo_i[:, cols_g].T)
        yr_blk = np.concatenate(
            [wotr.reshape(HLOC, HD, 1024), -woti.reshape(HLOC, HD, 1024)],
            axis=1).reshape(512, 1024)
        yi_blk = np.concatenate(
            [woti.reshape(HLOC, HD, 1024), wotr.reshape(HLOC, HD, 1024)],
            axis=1).reshape(512, 1024)
        wo_host = _round_f32r(np.concatenate([yr_blk, yi_blk], axis=0))

        qkb = np.zeros((P, 8), np.float32)
        for h in range(HLOC):
            qrows = slice(g * 256 + h * HD, g * 256 + (h + 1) * HD)
            krows = slice(DIM + g * 256 + h * HD, DIM + g * 256 + (h + 1) * HD)
            qkb[:, h] = np.concatenate([bdiff[qrows], bsum[qrows]])
            qkb[:, 4 + h] = np.concatenate([bdiff[krows], bsum[krows]])
        vbias = np.broadcast_to(
            np.concatenate([bdiff[vrows], bsum[vrows]]), (P, 512)).copy()
        if g == 0:
            obias = np.concatenate(
                [np.broadcast_to(bo_r - bo_i, (P, 1024)),
                 np.broadcast_to(bo_r + bo_i, (P, 1024))], axis=1).astype(np.float32)
        else:
            obias = np.zeros((P, 2048), np.float32)
        per_g.append((wq_host, wk_host, wv_host, wo_host, qkb, vbias,
                      np.ascontiguousarray(obias)))

    in_maps = []
    for core in range(8):
        b, g = divmod(core, G)
        wq_host, wk_host, wv_host, wo_host, qkb, vbias, obias = per_g[g]
        in_maps.append({
            "xs": xs_by_b[b], "wq": wq_host, "wk": wk_host, "wv": wv_host,
            "wo": wo_host, "qk_bias": qkb, "vbias": vbias, "obias": obias,
        })
    return in_maps


def _get_program():
    if "nc" not in _CACHE:
        _CACHE["nc"] = _build_program()
    return _CACHE["nc"]


def run(inputs: dict, trace: bool = False):
    """Returns (output, BassKernelResults)."""
    nc = _get_program()
    in_maps = _prepare_in_maps(**inputs)
    res = run_bass_kernel_spmd(nc, in_maps, list(range(8)), trace=trace)
    out = np.zeros((B, N, DIM, 2), np.float64)
    for core in range(8):
        b = core // G
        out[b, :, :, 0] += res.results[core]["yr"]
        out[b, :, :, 1] += res.results[core]["yi"]
    return out.astype(np.float32), res


def kernel(**inputs) -> np.ndarray:
    out, _ = run(inputs)
    return out


# revision 22
# speedup vs baseline: 1.1776x; 1.0042x over previous
"""ComplexAttention Trainium2 kernel (Bass/Tile, SPMD over 8 NeuronCores).

Problem: complex-valued multi-head attention (B=2, N=2048, DIM=1024, 16 heads,
head_dim 64), fp32. See the reference:
  qkv = complex_linear(x, wqkv)           # 4 real matmuls + bias
  attn = softmax(Re(q . conj(k)) * scale) # contract head_dim AND real/imag
  out  = attn @ v (both channels)
  y    = complex_linear(out, wo)

Sharding (8 cores): data-parallel over batch (2) x tensor-parallel over head
groups (4 groups x 4 heads). Each core computes q/k/v for its 4 heads, full
attention for those heads, and a PARTIAL output projection (contraction over
its 256 of the 1024 concat features). The host sums the 4 partials per batch.

Numerics: all matmuls run as float32r (e8m11, ~2.4e-4 rel) at full PE rate
(1 cycle/row for moving dim >= 256 - 4x faster than plain fp32). Inputs are
pre-rounded to f32r on the host; PSUM accumulation is fp32.

Device data layouts (per core):
  xs    (2048, 2048) f32r  rows = [x_real.T (1024); x_imag.T (1024)], cols=tokens
  wq/wk (1024, 1024) f32r  cols = per head h: [A_h (128) | B_h (128)] where
                           A_h = [w_r_h; w_i_h].T, B_h = [-w_i_h; w_r_h].T
                           -> feature-major psum tiles [ (re_h 64; im_h 64), n ]
  wv    (1024, 768)  f32r  cols = [wv_r.T | -wv_i.T | wv_i.T] (token-major V)
  wo    (1024, 1024) f32r  rows 0:512 -> y_real coeffs, 512:1024 -> y_imag;
                           row order h*128 + c*64 + d matches AO layout
  qk_bias (128, 8) f32     per-partition bias columns [q h0..h3, k h0..h3]
  vbias (128, 512) f32     broadcast rows, cols [re bias 256 | im bias 256]
  obias (128, 2048) f32    broadcast rows [y_re 1024 | y_im 1024]; zero on g>0
Outputs: yr, yi (2048, 1024) f32 partial projections.

Softmax needs no max subtraction: scores are ~N(0,1) here (max |s| << 80).
Row sums come from a ones-column matmul chain; normalization is deferred to
after attn@V (divide the 128-row per-head output, not the 2048-row E matrix).

Pass 1 computes Q, K (feature-major) and V (token-major) for all tokens,
streaming x in 256-token blocks (f32r full-rate moving dim). Pass 2 runs
attention + projection with 512-wide tiles to amortize the per-matmul f32r
weight-load (~134ns).
"""

from contextlib import ExitStack

import numpy as np

import concourse.bacc as bacc
import concourse.mybir as mybir
import concourse.tile as tile
from concourse.bass_utils import run_bass_kernel_spmd

F32 = mybir.dt.float32
F32R = mybir.dt.float32r

B = 2
N = 2048
DIM = 1024
HEADS = 16
HD = 64
G = 4          # head groups (tensor-parallel factor)
HLOC = HEADS // G
SCALE = 1.0 / 8.0
NB1 = 256      # pass-1 token block
NB2 = 512      # pass-2 token block
DT = DIM // 128  # 8 contraction tiles per 1024
P = 128
MT = N // P    # 16 m-tiles

_CACHE = {}


def _round_f32r(a: np.ndarray) -> np.ndarray:
    """Round-to-nearest-even fp32 -> fp32r (e8m11: low 12 mantissa bits zero)."""
    v = np.ascontiguousarray(a, dtype=np.float32).view(np.uint32).copy()
    lsb = (v >> np.uint32(12)) & np.uint32(1)
    v = v + np.uint32(0x7FF) + lsb
    v &= np.uint32(0xFFFFF000)
    return v.view(np.float32)


def _build_program():
    nc = bacc.Bacc("TRN2", target_bir_lowering=False, debug=False, num_devices=8,
                   dynamic_dma_scratch_size=2048)

    xs = nc.dram_tensor("xs", [2 * DIM, N], F32R, kind="ExternalInput").ap()
    wq = nc.dram_tensor("wq", [DIM, 1024], F32R, kind="ExternalInput").ap()
    wk = nc.dram_tensor("wk", [DIM, 1024], F32R, kind="ExternalInput").ap()
    wv = nc.dram_tensor("wv", [DIM, 768], F32R, kind="ExternalInput").ap()
    wo = nc.dram_tensor("wo", [1024, 1024], F32R, kind="ExternalInput").ap()
    qkb_d = nc.dram_tensor("qk_bias", [P, 8], F32, kind="ExternalInput").ap()
    vb_d = nc.dram_tensor("vbias", [P, 512], F32, kind="ExternalInput").ap()
    ob_d = nc.dram_tensor("obias", [P, 2048], F32, kind="ExternalInput").ap()
    yr = nc.dram_tensor("yr", [N, 1024], F32, kind="ExternalOutput").ap()
    yi = nc.dram_tensor("yi", [N, 1024], F32, kind="ExternalOutput").ap()

    xs_r = xs.rearrange("(t p) m -> p t m", p=P)   # [128, 16, 2048]
    wq_r = wq.rearrange("(t p) c -> p t c", p=P)   # [128, 8, 1024]
    wk_r = wk.rearrange("(t p) c -> p t c", p=P)
    wv_r = wv.rearrange("(t p) c -> p t c", p=P)   # [128, 8, 768]
    wo_r = wo.rearrange("(t p) c -> p t c", p=P)   # [128, 8, 1024]

    with tile.TileContext(nc) as tc, ExitStack() as ctx:
        const = ctx.enter_context(tc.tile_pool(name="const", bufs=1))
        kvp = ctx.enter_context(tc.tile_pool(name="kv", bufs=1))

        onesc_f = const.tile([P, 1], F32)
        ones_col = const.tile([P, 1], F32R)
        nc.vector.memset(onesc_f[:], 1.0)
        nc.vector.tensor_copy(ones_col[:], onesc_f[:])
        qkb = const.tile([P, 8], F32)
        nc.sync.dma_start(out=qkb[:], in_=qkb_d[:])

        Q_sb = kvp.tile([P, HLOC, N], F32R)          # [comps, head, n]
        K_sb = kvp.tile([P, HLOC, N], F32R)          # [comps, head, m]
        # [m%128, mtile, head, c*64+d] - per-head [Vr_h | Vi_h] contiguous so
        # the PV stationary slice is a single free dim
        V_sb = kvp.tile([P, MT, HLOC, 128], F32R)

        p1 = ExitStack()
        xsp = p1.enter_context(tc.tile_pool(name="xs", bufs=3))
        # ---------------- pass 1: V first (small weight prereq), then K/Q ---
        # wk preloads during the V pass; wq loads during the first K chains
        # (Q chains lag K by one m-block so the load hides).
        with tc.tile_pool(name="wkp", bufs=1) as wkp:
            wk_t = []
            for dt in range(DT):
                wkt = wkp.tile([P, 1024], F32R, tag=f"wk{dt}", name=f"wk{dt}")
                wk_t.append(wkt)

            with tc.tile_pool(name="wvv", bufs=1) as wvvp:
                wv_t = []
                for dt in range(DT):
                    wvt = wvvp.tile([P, 768], F32R, tag=f"wv{dt}", name=f"wv{dt}")
                    nc.sync.dma_start(out=wvt[:], in_=wv_r[:, dt, :])
                    wv_t.append(wvt)
                vb = wvvp.tile([P, 512], F32)
                nc.sync.dma_start(out=vb[:], in_=vb_d[:])
                # first two x blocks ahead of the wk prefetch in the DMA queue
                xt_pre = []
                for mb in range(2):
                    xt = xsp.tile([P, 2 * DT, NB1], F32R, tag="xs",
                                  name=f"xtpre{mb}")
                    nc.sync.dma_start(
                        out=xt[:], in_=xs_r[:, :, mb * NB1:(mb + 1) * NB1])
                    xt_pre.append(xt)
                for dt in range(DT):
                    nc.sync.dma_start(out=wk_t[dt][:], in_=wk_r[:, dt, :])

                with tc.tile_pool(name="p1psb", bufs=6, space="PSUM") as pmm:
                    for mb in range(N // NB1):
                        if mb < 2:
                            xt = xt_pre[mb]
                        else:
                            xt = xsp.tile([P, 2 * DT, NB1], F32R, tag="xs")
                            nc.sync.dma_start(
                                out=xt[:], in_=xs_r[:, :, mb * NB1:(mb + 1) * NB1])
                        for mt in range(NB1 // P):
                            mtg = mb * (NB1 // P) + mt
                            for c in range(2):
                                kind_a = 0 if c == 0 else 2   # wv_r.T / wv_i.T
                                kind_b = 1 if c == 0 else 0   # -wv_i.T / wv_r.T
                                ps = pmm.tile([P, NB2], F32, tag="mm")
                                for dt in range(DT):
                                    nc.tensor.matmul(
                                        ps[:, :256], xt[:, dt, mt * P:(mt + 1) * P],
                                        wv_t[dt][:, kind_a * 256:(kind_a + 1) * 256],
                                        start=(dt == 0), stop=False)
                                for dt in range(DT):
                                    nc.tensor.matmul(
                                        ps[:, :256],
                                        xt[:, DT + dt, mt * P:(mt + 1) * P],
                                        wv_t[dt][:, kind_b * 256:(kind_b + 1) * 256],
                                        start=False, stop=(dt == DT - 1))
                                nc.vector.tensor_add(
                                    V_sb[:, mtg, :, c * HD:(c + 1) * HD],
                                    ps[:, :256].rearrange("p (h f) -> p h f", f=HD),
                                    vb[:, c * 256:(c + 1) * 256].rearrange(
                                        "p (h f) -> p h f", f=HD))

            # ---- K/Q: third x stream; Q lags K by one block -----------------
            with tc.tile_pool(name="wqp2", bufs=1) as wqp2, \
                 tc.tile_pool(name="p1ps", bufs=6, space="PSUM") as pmm:
                wq_t = []
                for dt in range(DT):
                    wqt = wqp2.tile([P, 1024], F32R, tag=f"wq{dt}", name=f"wq{dt}")
                    nc.sync.dma_start(out=wqt[:], in_=wq_r[:, dt, :])
                    wq_t.append(wqt)

                def kq_chains(w_t, sb, bcol, h, xt, mb):
                    ps = pmm.tile([P, NB2], F32, tag="mm")
                    for dt in range(DT):
                        nc.tensor.matmul(
                            ps[:, :NB1], w_t[dt][:, h * 256:h * 256 + 128],
                            xt[:, dt, :], start=(dt == 0), stop=False)
                    for dt in range(DT):
                        nc.tensor.matmul(
                            ps[:, :NB1], w_t[dt][:, h * 256 + 128:h * 256 + 256],
                            xt[:, DT + dt, :], start=False, stop=(dt == DT - 1))
                    nc.vector.tensor_scalar_add(
                        sb[:, h, mb * NB1:(mb + 1) * NB1], ps[:, :NB1],
                        qkb[:, bcol + h:bcol + h + 1])

                prev_xt = None
                for mb in range(N // NB1):
                    xt = xsp.tile([P, 2 * DT, NB1], F32R, tag="xs")
                    nc.sync.dma_start(
                        out=xt[:], in_=xs_r[:, :, mb * NB1:(mb + 1) * NB1])
                    for h in range(HLOC):
                        kq_chains(wk_t, K_sb, 4, h, xt, mb)
                    if prev_xt is not None:
                        for h in range(HLOC):
                            kq_chains(wq_t, Q_sb, 0, h, prev_xt[0], prev_xt[1])
                    prev_xt = (xt, mb)
                for h in range(HLOC):
                    kq_chains(wq_t, Q_sb, 0, h, prev_xt[0], prev_xt[1])

        p1.close()

        # ---------------- pass 2: attention + partial out-projection -------
        with tc.tile_pool(name="w2", bufs=1) as w2p, \
             tc.tile_pool(name="pmm2", bufs=3, space="PSUM") as pmm, \
             tc.tile_pool(name="ppv", bufs=1, space="PSUM") as ppv, \
             tc.tile_pool(name="prs", bufs=1, space="PSUM") as prs, \
             tc.tile_pool(name="epool", bufs=4) as ep, \
             tc.tile_pool(name="aop", bufs=2) as aop, \
             tc.tile_pool(name="rbp", bufs=4) as rbp, \
             tc.tile_pool(name="outp", bufs=4) as outp:
            wo_sb = w2p.tile([P, 8, 1024], F32R)
            ob = w2p.tile([P, 2048], F32)
            nc.sync.dma_start(out=wo_sb[:], in_=wo_r[:])
            nc.sync.dma_start(out=ob[:], in_=ob_d[:])

            def emit_proj(ao, nb):
                for ns in range(NB2 // P):
                    for half in range(2):
                        for ri, (ydram, bofs) in enumerate(((yr, 0), (yi, 1024))):
                            pp = pmm.tile([P, 512], F32, tag="mm")
                            for t in range(4):
                                nc.tensor.matmul(
                                    pp[:], ao[:, t, ns * P:(ns + 1) * P],
                                    wo_sb[:, 4 * ri + t, half * 512:(half + 1) * 512],
                                    start=(t == 0), stop=(t == 3))
                            ot = outp.tile([P, 512], F32, tag="ot")
                            nc.vector.tensor_add(
                                ot[:], pp[:],
                                ob[:, bofs + half * 512:bofs + (half + 1) * 512])
                            nc.sync.dma_start(
                                out=ydram[nb * NB2 + ns * P:nb * NB2 + (ns + 1) * P,
                                          half * 512:(half + 1) * 512],
                                in_=ot[:])

            prev = None
            for nb in range(N // NB2):
                ao = aop.tile([P, HLOC, NB2], F32R)
                for h in range(HLOC):
                    pv = ppv.tile([P, NB2], F32, tag="pv")
                    rs = prs.tile([1, NB2], F32, tag="rs")
                    # software pipeline: scores one pair ahead of PV/rowsum so
                    # the PE never stalls on the exp latency
                    pend = None
                    for mtp in range(MT // 2):
                        mt0, mt1 = 2 * mtp, 2 * mtp + 1
                        ss = pmm.tile([P, 2 * NB2], F32, tag="mm")
                        nc.tensor.matmul(
                            ss[:, :NB2], K_sb[:, h, mt0 * P:(mt0 + 1) * P],
                            Q_sb[:, h, nb * NB2:(nb + 1) * NB2],
                            start=True, stop=True)
                        nc.tensor.matmul(
                            ss[:, NB2:], K_sb[:, h, mt1 * P:(mt1 + 1) * P],
                            Q_sb[:, h, nb * NB2:(nb + 1) * NB2],
                            start=True, stop=True)
                        e = ep.tile([P, 2 * NB2], F32R, tag="e")
                        nc.scalar.activation(
                            e[:], ss[:], mybir.ActivationFunctionType.Exp,
                            scale=SCALE)
                        if pend is not None:
                            pe, pmt = pend
                            nc.tensor.matmul(
                                pv[:], V_sb[:, 2 * pmt, h, :], pe[:, :NB2],
                                start=(pmt == 0), stop=False)
                            nc.tensor.matmul(
                                pv[:], V_sb[:, 2 * pmt + 1, h, :], pe[:, NB2:],
                                start=False, stop=False)
                            nc.tensor.matmul(
                                rs[:], ones_col[:], pe[:, :NB2],
                                start=(pmt == 0), stop=False)
                            nc.tensor.matmul(
                                rs[:], ones_col[:], pe[:, NB2:],
                                start=False, stop=False)
                        pend = (e, mtp)
                    pe, pmt = pend
                    nc.tensor.matmul(
                        pv[:], V_sb[:, 2 * pmt, h, :], pe[:, :NB2],
                        start=False, stop=False)
                    nc.tensor.matmul(
                        pv[:], V_sb[:, 2 * pmt + 1, h, :], pe[:, NB2:],
                        start=False, stop=True)
                    nc.tensor.matmul(
                        rs[:], ones_col[:], pe[:, :NB2],
                        start=False, stop=False)
                    nc.tensor.matmul(
                        rs[:], ones_col[:], pe[:, NB2:],
                        start=False, stop=True)
                    rsc = rbp.tile([1, NB2], F32, tag="rsc", bufs=2)
                    nc.vector.tensor_copy(rsc[:], rs[:])
                    # release pv early: stage unnormalized output into ao, then
                    # scale in place once the reciprocal lands
                    nc.vector.tensor_copy(ao[:, h, :], pv[:])
                    # the deferred projection goes first so its PSUM-freeing
                    # DVE adds are not stuck behind the 3.3us reciprocal
                    if h == 0 and prev is not None:
                        emit_proj(*prev)
                    rbr = rbp.tile([P, NB2], F32, tag="rbr")
                    nc.gpsimd.partition_broadcast(rbr[:], rsc[:])
                    rbs = rbp.tile([P, NB2], F32, tag="rb")
                    with nc.allow_low_precision(reason="softmax divisor"):
                        nc.vector.reciprocal(rbs[:], rbr[:])
                    nc.vector.tensor_mul(ao[:, h, :], ao[:, h, :], rbs[:])
                prev = (ao, nb)
            emit_proj(*prev)
    nc.compile()
    return nc


def _prepare_in_maps(x, wqkv_r, wqkv_i, bqkv_r, bqkv_i, wo_r, wo_i, bo_r, bo_i):
    x = np.asarray(x, np.float32)
    wqkv_r = np.asarray(wqkv_r, np.float32)
    wqkv_i = np.asarray(wqkv_i, np.float32)
    bqkv_r = np.asarray(bqkv_r, np.float32)
    bqkv_i = np.asarray(bqkv_i, np.float32)
    wo_r = np.asarray(wo_r, np.float32)
    wo_i = np.asarray(wo_i, np.float32)
    bo_r = np.asarray(bo_r, np.float32)
    bo_i = np.asarray(bo_i, np.float32)

    bdiff = bqkv_r - bqkv_i
    bsum = bqkv_r + bqkv_i

    xs_by_b = []
    for b in range(B):
        xs_by_b.append(_round_f32r(np.concatenate(
            [np.ascontiguousarray(x[b, :, :, 0].T),
             np.ascontiguousarray(x[b, :, :, 1].T)], axis=0)))

    per_g = []
    for g in range(G):
        # q/k weights: per head, stacked-complex A/B column blocks
        def head_cols(base):
            cols = []
            for h in range(HLOC):
                rows = slice(base + g * 256 + h * HD, base + g * 256 + (h + 1) * HD)
                a = np.concatenate([wqkv_r[rows], wqkv_i[rows]], axis=0).T
                bb = np.concatenate([-wqkv_i[rows], wqkv_r[rows]], axis=0).T
                cols.append(a)
                cols.append(bb)
            return _round_f32r(np.concatenate(cols, axis=1))  # (1024, 1024)

        wq_host = head_cols(0)
        wk_host = head_cols(DIM)
        vrows = slice(2 * DIM + g * 256, 2 * DIM + (g + 1) * 256)
        wv_host = _round_f32r(np.concatenate(
            [wqkv_r[vrows].T, -wqkv_i[vrows].T, wqkv_i[vrows].T], axis=1))

        cols_g = slice(g * 256, (g + 1) * 256)
        wotr = np.ascontiguousarray(wo_r[:, cols_g].T)   # (256 fi, 1024 fo)
        woti = np.ascontiguousarray(wo_i[:, cols_g].T)
        yr_blk = np.concatenate(
            [wotr.reshape(HLOC, HD, 1024), -woti.reshape(HLOC, HD, 1024)],
            axis=1).reshape(512, 1024)
        yi_blk = np.concatenate(
            [woti.reshape(HLOC, HD, 1024), wotr.reshape(HLOC, HD, 1024)],
            axis=1).reshape(512, 1024)
        wo_host = _round_f32r(np.concatenate([yr_blk, yi_blk], axis=0))

        qkb = np.zeros((P, 8), np.float32)
        for h in range(HLOC):
            qrows = slice(g * 256 + h * HD, g * 256 + (h + 1) * HD)
            krows = slice(DIM + g * 256 + h * HD, DIM + g * 256 + (h + 1) * HD)
            qkb[:, h] = np.concatenate([bdiff[qrows], bsum[qrows]])
            qkb[:, 4 + h] = np.concatenate([bdiff[krows], bsum[krows]])
        vbias = np.broadcast_to(
            np.concatenate([bdiff[vrows], bsum[vrows]]), (P, 512)).copy()
        if g == 0:
            obias = np.concatenate(
                [np.broadcast_to(bo_r - bo_i, (P, 1024)),
                 np.broadcast_to(bo_r + bo_i, (P, 1024))], axis=1).astype(np.float32)
        else:
            obias = np.zeros((P, 2048), np.float32)
        per_g.append((wq_host, wk_host, wv_host, wo_host, qkb, vbias,
                      np.ascontiguousarray(obias)))

    in_maps = []
    for core in range(8):
        b, g = divmod(core, G)
        wq_host, wk_host, wv_host, wo_host, qkb, vbias, obias = per_g[g]
        in_maps.append({
            "xs": xs_by_b[b], "wq": wq_host, "wk": wk_host, "wv": wv_host,
            "wo": wo_host, "qk_bias": qkb, "vbias": vbias, "obias": obias,
        })
    return in_maps


def _get_program():
    if "nc" not in _CACHE:
        _CACHE["nc"] = _build_program()
    return _CACHE["nc"]


def run(inputs: dict, trace: bool = False):
    """Returns (output, BassKernelResults)."""
    nc = _get_program()
    in_maps = _prepare_in_maps(**inputs)
    res = run_bass_kernel_spmd(nc, in_maps, list(range(8)), trace=trace)
    out = np.zeros((B, N, DIM, 2), np.float64)
    for core in range(8):
        b = core // G
        out[b, :, :, 0] += res.results[core]["yr"]
        out[b, :, :, 1] += res.results[core]["yi"]
    return out.astype(np.float32), res


def kernel(**inputs) -> np.ndarray:
    out, _ = run(inputs)
    return out


# revision 23
# speedup vs baseline: 1.1983x; 1.0176x over previous
"""ComplexAttention Trainium2 kernel (Bass/Tile, SPMD over 8 NeuronCores).

Problem: complex-valued multi-head attention (B=2, N=2048, DIM=1024, 16 heads,
head_dim 64), fp32. See the reference:
  qkv = complex_linear(x, wqkv)           # 4 real matmuls + bias
  attn = softmax(Re(q . conj(k)) * scale) # contract head_dim AND real/imag
  out  = attn @ v (both channels)
  y    = complex_linear(out, wo)

Sharding (8 cores): data-parallel over batch (2) x tensor-parallel over head
groups (4 groups x 4 heads). Each core computes q/k/v for its 4 heads, full
attention for those heads, and a PARTIAL output projection (contraction over
its 256 of the 1024 concat features). The host sums the 4 partials per batch.

Numerics: all matmuls run as float32r (e8m11, ~2.4e-4 rel) at full PE rate
(1 cycle/row for moving dim >= 256 - 4x faster than plain fp32). Inputs are
pre-rounded to f32r on the host; PSUM accumulation is fp32.

Device data layouts (per core):
  xs    (2048, 2048) f32r  rows = [x_real.T (1024); x_imag.T (1024)], cols=tokens
  wq/wk (1024, 1024) f32r  cols = per head h: [A_h (128) | B_h (128)] where
                           A_h = [w_r_h; w_i_h].T, B_h = [-w_i_h; w_r_h].T
                           -> feature-major psum tiles [ (re_h 64; im_h 64), n ]
  wv    (1024, 768)  f32r  cols = [wv_r.T | -wv_i.T | wv_i.T] (token-major V)
  wo    (1024, 1024) f32r  rows 0:512 -> y_real coeffs, 512:1024 -> y_imag;
                           row order h*128 + c*64 + d matches AO layout
  qk_bias (128, 8) f32     per-partition bias columns [q h0..h3, k h0..h3]
  vbias (128, 512) f32     broadcast rows, cols [re bias 256 | im bias 256]
  obias (128, 2048) f32    broadcast rows [y_re 1024 | y_im 1024]; zero on g>0
Outputs: yr, yi (2048, 1024) f32 partial projections.

Softmax needs no max subtraction: scores are ~N(0,1) here (max |s| << 80).
Row sums come from a ones-column matmul chain; normalization is deferred to
after attn@V (divide the 128-row per-head output, not the 2048-row E matrix).

Pass 1 computes Q, K (feature-major) and V (token-major) for all tokens,
streaming x in 256-token blocks (f32r full-rate moving dim). Pass 2 runs
attention + projection with 512-wide tiles to amortize the per-matmul f32r
weight-load (~134ns).
"""

from contextlib import ExitStack

import numpy as np

import concourse.bacc as bacc
import concourse.mybir as mybir
import concourse.tile as tile
from concourse.bass_utils import run_bass_kernel_spmd

F32 = mybir.dt.float32
F32R = mybir.dt.float32r

B = 2
N = 2048
DIM = 1024
HEADS = 16
HD = 64
G = 4          # head groups (tensor-parallel factor)
HLOC = HEADS // G
SCALE = 1.0 / 8.0
NB1 = 256      # pass-1 token block
NB2 = 512      # pass-2 token block
DT = DIM // 128  # 8 contraction tiles per 1024
P = 128
MT = N // P    # 16 m-tiles

_CACHE = {}


def _round_f32r(a: np.ndarray) -> np.ndarray:
    """Round-to-nearest-even fp32 -> fp32r (e8m11: low 12 mantissa bits zero)."""
    v = np.ascontiguousarray(a, dtype=np.float32).view(np.uint32).copy()
    lsb = (v >> np.uint32(12)) & np.uint32(1)
    v = v + np.uint32(0x7FF) + lsb
    v &= np.uint32(0xFFFFF000)
    return v.view(np.float32)


def _build_program():
    nc = bacc.Bacc("TRN2", target_bir_lowering=False, debug=False, num_devices=8,
                   dynamic_dma_scratch_size=2048)

    xs = nc.dram_tensor("xs", [N // NB1, P, 2 * DT, NB1], F32R,
                        kind="ExternalInput").ap()
    wq = nc.dram_tensor("wq", [DIM, 1024], F32R, kind="ExternalInput").ap()
    wk = nc.dram_tensor("wk", [DIM, 1024], F32R, kind="ExternalInput").ap()
    wv = nc.dram_tensor("wv", [DIM, 768], F32R, kind="ExternalInput").ap()
    wo = nc.dram_tensor("wo", [1024, 1024], F32R, kind="ExternalInput").ap()
    qkb_d = nc.dram_tensor("qk_bias", [P, 8], F32, kind="ExternalInput").ap()
    vb_d = nc.dram_tensor("vbias", [P, 512], F32, kind="ExternalInput").ap()
    ob_d = nc.dram_tensor("obias", [P, 2048], F32, kind="ExternalInput").ap()
    yr = nc.dram_tensor("yr", [N, 1024], F32, kind="ExternalOutput").ap()
    yi = nc.dram_tensor("yi", [N, 1024], F32, kind="ExternalOutput").ap()

    wq_r = wq.rearrange("(t p) c -> p t c", p=P)   # [128, 8, 1024]
    wk_r = wk.rearrange("(t p) c -> p t c", p=P)
    wv_r = wv.rearrange("(t p) c -> p t c", p=P)   # [128, 8, 768]
    wo_r = wo.rearrange("(t p) c -> p t c", p=P)   # [128, 8, 1024]

    with tile.TileContext(nc) as tc, ExitStack() as ctx:
        const = ctx.enter_context(tc.tile_pool(name="const", bufs=1))
        kvp = ctx.enter_context(tc.tile_pool(name="kv", bufs=1))

        onesc_f = const.tile([P, 1], F32)
        ones_col = const.tile([P, 1], F32R)
        nc.vector.memset(onesc_f[:], 1.0)
        nc.vector.tensor_copy(ones_col[:], onesc_f[:])
        qkb = const.tile([P, 8], F32)
        nc.sync.dma_start(out=qkb[:], in_=qkb_d[:])

        Q_sb = kvp.tile([P, HLOC, N], F32R)          # [comps, head, n]
        K_sb = kvp.tile([P, HLOC, N], F32R)          # [comps, head, m]
        # [m%128, mtile, head, c*64+d] - per-head [Vr_h | Vi_h] contiguous so
        # the PV stationary slice is a single free dim
        V_sb = kvp.tile([P, MT, HLOC, 128], F32R)

        p1 = ExitStack()
        xsp = p1.enter_context(tc.tile_pool(name="xs", bufs=3))
        # ---------------- pass 1: V first (small weight prereq), then K/Q ---
        # wk preloads during the V pass; wq loads during the first K chains
        # (Q chains lag K by one m-block so the load hides).
        with tc.tile_pool(name="wkp", bufs=1) as wkp:
            wk_t = []
            for dt in range(DT):
                wkt = wkp.tile([P, 1024], F32R, tag=f"wk{dt}", name=f"wk{dt}")
                wk_t.append(wkt)

            with tc.tile_pool(name="wvv", bufs=1) as wvvp:
                wv_t = []
                for dt in range(DT):
                    wvt = wvvp.tile([P, 768], F32R, tag=f"wv{dt}", name=f"wv{dt}")
                    nc.sync.dma_start(out=wvt[:], in_=wv_r[:, dt, :])
                    wv_t.append(wvt)
                vb = wvvp.tile([P, 512], F32)
                nc.sync.dma_start(out=vb[:], in_=vb_d[:])
                # first two x blocks ahead of the wk prefetch in the DMA queue
                xt_pre = []
                for mb in range(2):
                    xt = xsp.tile([P, 2 * DT, NB1], F32R, tag="xs",
                                  name=f"xtpre{mb}")
                    nc.sync.dma_start(out=xt[:], in_=xs[mb])
                    xt_pre.append(xt)
                for dt in range(DT):
                    nc.sync.dma_start(out=wk_t[dt][:], in_=wk_r[:, dt, :])

                with tc.tile_pool(name="p1psb", bufs=6, space="PSUM") as pmm:
                    for mb in range(N // NB1):
                        if mb < 2:
                            xt = xt_pre[mb]
                        else:
                            xt = xsp.tile([P, 2 * DT, NB1], F32R, tag="xs")
                            nc.sync.dma_start(out=xt[:], in_=xs[mb])
                        for mt in range(NB1 // P):
                            mtg = mb * (NB1 // P) + mt
                            for c in range(2):
                                kind_a = 0 if c == 0 else 2   # wv_r.T / wv_i.T
                                kind_b = 1 if c == 0 else 0   # -wv_i.T / wv_r.T
                                ps = pmm.tile([P, NB2], F32, tag="mm")
                                for dt in range(DT):
                                    nc.tensor.matmul(
                                        ps[:, :256], xt[:, dt, mt * P:(mt + 1) * P],
                                        wv_t[dt][:, kind_a * 256:(kind_a + 1) * 256],
                                        start=(dt == 0), stop=False)
                                for dt in range(DT):
                                    nc.tensor.matmul(
                                        ps[:, :256],
                                        xt[:, DT + dt, mt * P:(mt + 1) * P],
                                        wv_t[dt][:, kind_b * 256:(kind_b + 1) * 256],
                                        start=False, stop=(dt == DT - 1))
                                nc.vector.tensor_add(
                                    V_sb[:, mtg, :, c * HD:(c + 1) * HD],
                                    ps[:, :256].rearrange("p (h f) -> p h f", f=HD),
                                    vb[:, c * 256:(c + 1) * 256].rearrange(
                                        "p (h f) -> p h f", f=HD))

            # ---- K/Q: third x stream; Q lags K by one block -----------------
            with tc.tile_pool(name="wqp2", bufs=1) as wqp2, \
                 tc.tile_pool(name="p1ps", bufs=6, space="PSUM") as pmm:
                wq_t = []
                for dt in range(DT):
                    wqt = wqp2.tile([P, 1024], F32R, tag=f"wq{dt}", name=f"wq{dt}")
                    nc.sync.dma_start(out=wqt[:], in_=wq_r[:, dt, :])
                    wq_t.append(wqt)

                def kq_chains(w_t, sb, bcol, h, xt, mb):
                    ps = pmm.tile([P, NB2], F32, tag="mm")
                    for dt in range(DT):
                        nc.tensor.matmul(
                            ps[:, :NB1], w_t[dt][:, h * 256:h * 256 + 128],
                            xt[:, dt, :], start=(dt == 0), stop=False)
                    for dt in range(DT):
                        nc.tensor.matmul(
                            ps[:, :NB1], w_t[dt][:, h * 256 + 128:h * 256 + 256],
                            xt[:, DT + dt, :], start=False, stop=(dt == DT - 1))
                    nc.vector.tensor_scalar_add(
                        sb[:, h, mb * NB1:(mb + 1) * NB1], ps[:, :NB1],
                        qkb[:, bcol + h:bcol + h + 1])

                prev_xt = None
                for mb in range(N // NB1):
                    xt = xsp.tile([P, 2 * DT, NB1], F32R, tag="xs")
                    nc.sync.dma_start(out=xt[:], in_=xs[mb])
                    for h in range(HLOC):
                        kq_chains(wk_t, K_sb, 4, h, xt, mb)
                    if prev_xt is not None:
                        for h in range(HLOC):
                            kq_chains(wq_t, Q_sb, 0, h, prev_xt[0], prev_xt[1])
                    prev_xt = (xt, mb)
                for h in range(HLOC):
                    kq_chains(wq_t, Q_sb, 0, h, prev_xt[0], prev_xt[1])

        p1.close()

        # ---------------- pass 2: attention + partial out-projection -------
        with tc.tile_pool(name="w2", bufs=1) as w2p, \
             tc.tile_pool(name="pmm2", bufs=3, space="PSUM") as pmm, \
             tc.tile_pool(name="ppv", bufs=1, space="PSUM") as ppv, \
             tc.tile_pool(name="prs", bufs=1, space="PSUM") as prs, \
             tc.tile_pool(name="epool", bufs=4) as ep, \
             tc.tile_pool(name="aop", bufs=2) as aop, \
             tc.tile_pool(name="rbp", bufs=4) as rbp, \
             tc.tile_pool(name="outp", bufs=4) as outp:
            wo_sb = w2p.tile([P, 8, 1024], F32R)
            ob = w2p.tile([P, 2048], F32)
            nc.sync.dma_start(out=wo_sb[:], in_=wo_r[:])
            nc.sync.dma_start(out=ob[:], in_=ob_d[:])

            def emit_proj(ao, nb):
                for ns in range(NB2 // P):
                    for half in range(2):
                        for ri, (ydram, bofs) in enumerate(((yr, 0), (yi, 1024))):
                            pp = pmm.tile([P, 512], F32, tag="mm")
                            for t in range(4):
                                nc.tensor.matmul(
                                    pp[:], ao[:, t, ns * P:(ns + 1) * P],
                                    wo_sb[:, 4 * ri + t, half * 512:(half + 1) * 512],
                                    start=(t == 0), stop=(t == 3))
                            ot = outp.tile([P, 512], F32, tag="ot")
                            nc.vector.tensor_add(
                                ot[:], pp[:],
                                ob[:, bofs + half * 512:bofs + (half + 1) * 512])
                            nc.sync.dma_start(
                                out=ydram[nb * NB2 + ns * P:nb * NB2 + (ns + 1) * P,
                                          half * 512:(half + 1) * 512],
                                in_=ot[:])

            prev = None
            for nb in range(N // NB2):
                ao = aop.tile([P, HLOC, NB2], F32R)
                for h in range(HLOC):
                    pv = ppv.tile([P, NB2], F32, tag="pv")
                    rs = prs.tile([1, NB2], F32, tag="rs")
                    # software pipeline: scores one pair ahead of PV/rowsum so
                    # the PE never stalls on the exp latency
                    pend = None
                    for mtp in range(MT // 2):
                        mt0, mt1 = 2 * mtp, 2 * mtp + 1
                        ss = pmm.tile([P, 2 * NB2], F32, tag="mm")
                        nc.tensor.matmul(
                            ss[:, :NB2], K_sb[:, h, mt0 * P:(mt0 + 1) * P],
                            Q_sb[:, h, nb * NB2:(nb + 1) * NB2],
                            start=True, stop=True)
                        nc.tensor.matmul(
                            ss[:, NB2:], K_sb[:, h, mt1 * P:(mt1 + 1) * P],
                            Q_sb[:, h, nb * NB2:(nb + 1) * NB2],
                            start=True, stop=True)
                        e = ep.tile([P, 2 * NB2], F32R, tag="e")
                        nc.scalar.activation(
                            e[:], ss[:], mybir.ActivationFunctionType.Exp,
                            scale=SCALE)
                        if pend is not None:
                            pe, pmt = pend
                            nc.tensor.matmul(
                                pv[:], V_sb[:, 2 * pmt, h, :], pe[:, :NB2],
                                start=(pmt == 0), stop=False)
                            nc.tensor.matmul(
                                pv[:], V_sb[:, 2 * pmt + 1, h, :], pe[:, NB2:],
                                start=False, stop=False)
                            nc.tensor.matmul(
                                rs[:], ones_col[:], pe[:, :NB2],
                                start=(pmt == 0), stop=False)
                            nc.tensor.matmul(
                                rs[:], ones_col[:], pe[:, NB2:],
                                start=False, stop=False)
                        pend = (e, mtp)
                    pe, pmt = pend
                    nc.tensor.matmul(
                        pv[:], V_sb[:, 2 * pmt, h, :], pe[:, :NB2],
                        start=False, stop=False)
                    nc.tensor.matmul(
                        pv[:], V_sb[:, 2 * pmt + 1, h, :], pe[:, NB2:],
                        start=False, stop=True)
                    nc.tensor.matmul(
                        rs[:], ones_col[:], pe[:, :NB2],
                        start=False, stop=False)
                    nc.tensor.matmul(
                        rs[:], ones_col[:], pe[:, NB2:],
                        start=False, stop=True)
                    rsc = rbp.tile([1, NB2], F32, tag="rsc", bufs=2)
                    nc.vector.tensor_copy(rsc[:], rs[:])
                    # release pv early: stage unnormalized output into ao, then
                    # scale in place once the reciprocal lands
                    nc.vector.tensor_copy(ao[:, h, :], pv[:])
                    # the deferred projection goes first so its PSUM-freeing
                    # DVE adds are not stuck behind the 3.3us reciprocal
                    if h == 0 and prev is not None:
                        emit_proj(*prev)
                    rbr = rbp.tile([P, NB2], F32, tag="rbr")
                    nc.gpsimd.partition_broadcast(rbr[:], rsc[:])
                    rbs = rbp.tile([P, NB2], F32, tag="rb")
                    with nc.allow_low_precision(reason="softmax divisor"):
                        nc.vector.reciprocal(rbs[:], rbr[:])
                    nc.vector.tensor_mul(ao[:, h, :], ao[:, h, :], rbs[:])
                prev = (ao, nb)
            emit_proj(*prev)
    nc.compile()
    return nc


def _prepare_in_maps(x, wqkv_r, wqkv_i, bqkv_r, bqkv_i, wo_r, wo_i, bo_r, bo_i):
    x = np.asarray(x, np.float32)
    wqkv_r = np.asarray(wqkv_r, np.float32)
    wqkv_i = np.asarray(wqkv_i, np.float32)
    bqkv_r = np.asarray(bqkv_r, np.float32)
    bqkv_i = np.asarray(bqkv_i, np.float32)
    wo_r = np.asarray(wo_r, np.float32)
    wo_i = np.asarray(wo_i, np.float32)
    bo_r = np.asarray(bo_r, np.float32)
    bo_i = np.asarray(bo_i, np.float32)

    bdiff = bqkv_r - bqkv_i
    bsum = bqkv_r + bqkv_i

    xs_by_b = []
    for b in range(B):
        xsb = np.concatenate(
            [np.ascontiguousarray(x[b, :, :, 0].T),
             np.ascontiguousarray(x[b, :, :, 1].T)], axis=0)   # (2048 d2, 2048 m)
        xsb = xsb.reshape(2 * DT, P, N // NB1, NB1).transpose(2, 1, 0, 3)
        xs_by_b.append(_round_f32r(np.ascontiguousarray(xsb)))

    per_g = []
    for g in range(G):
        # q/k weights: per head, stacked-complex A/B column blocks
        def head_cols(base):
            cols = []
            for h in range(HLOC):
                rows = slice(base + g * 256 + h * HD, base + g * 256 + (h + 1) * HD)
                a = np.concatenate([wqkv_r[rows], wqkv_i[rows]], axis=0).T
                bb = np.concatenate([-wqkv_i[rows], wqkv_r[rows]], axis=0).T
                cols.append(a)
                cols.append(bb)
            return _round_f32r(np.concatenate(cols, axis=1))  # (1024, 1024)

        wq_host = head_cols(0)
        wk_host = head_cols(DIM)
        vrows = slice(2 * DIM + g * 256, 2 * DIM + (g + 1) * 256)
        wv_host = _round_f32r(np.concatenate(
            [wqkv_r[vrows].T, -wqkv_i[vrows].T, wqkv_i[vrows].T], axis=1))

        cols_g = slice(g * 256, (g + 1) * 256)
        wotr = np.ascontiguousarray(wo_r[:, cols_g].T)   # (256 fi, 1024 fo)
        woti = np.ascontiguousarray(wo_i[:, cols_g].T)
        yr_blk = np.concatenate(
            [wotr.reshape(HLOC, HD, 1024), -woti.reshape(HLOC, HD, 1024)],
            axis=1).reshape(512, 1024)
        yi_blk = np.concatenate(
            [woti.reshape(HLOC, HD, 1024), wotr.reshape(HLOC, HD, 1024)],
            axis=1).reshape(512, 1024)
        wo_host = _round_f32r(np.concatenate([yr_blk, yi_blk], axis=0))

        qkb = np.zeros((P, 8), np.float32)
        for h in range(HLOC):
            qrows = slice(g * 256 + h * HD, g * 256 + (h + 1) * HD)
            krows = slice(DIM + g * 256 + h * HD, DIM + g * 256 + (h + 1) * HD)
            qkb[:, h] = np.concatenate([bdiff[qrows], bsum[qrows]])
            qkb[:, 4 + h] = np.concatenate([bdiff[krows], bsum[krows]])
        vbias = np.broadcast_to(
            np.concatenate([bdiff[vrows], bsum[vrows]]), (P, 512)).copy()
        if g == 0:
            obias = np.concatenate(
                [np.broadcast_to(bo_r - bo_i, (P, 1024)),
                 np.broadcast_to(bo_r + bo_i, (P, 1024))], axis=1).astype(np.float32)
        else:
            obias = np.zeros((P, 2048), np.float32)
        per_g.append((wq_host, wk_host, wv_host, wo_host, qkb, vbias,
                      np.ascontiguousarray(obias)))

    in_maps = []
    for core in range(8):
        b, g = divmod(core, G)
        wq_host, wk_host, wv_host, wo_host, qkb, vbias, obias = per_g[g]
        in_maps.append({
            "xs": xs_by_b[b], "wq": wq_host, "wk": wk_host, "wv": wv_host,
            "wo": wo_host, "qk_bias": qkb, "vbias": vbias, "obias": obias,
        })
    return in_maps


def _get_program():
    if "nc" not in _CACHE:
        _CACHE["nc"] = _build_program()
    return _CACHE["nc"]


def run(inputs: dict, trace: bool = False):
    """Returns (output, BassKernelResults)."""
    nc = _get_program()
    in_maps = _prepare_in_maps(**inputs)
    res = run_bass_kernel_spmd(nc, in_maps, list(range(8)), trace=trace)
    out = np.zeros((B, N, DIM, 2), np.float64)
    for core in range(8):
        b = core // G
        out[b, :, :, 0] += res.results[core]["yr"]
        out[b, :, :, 1] += res.results[core]["yi"]
    return out.astype(np.float32), res


def kernel(**inputs) -> np.ndarray:
    out, _ = run(inputs)
    return out
